# revision 34
# baseline (speedup 1.0000x reference)
"""Trainium2 Bass kernel for agent-attention (AAGA): 8-core data-parallel over batch.

Math (per batch b):
  qkv = x @ W_qkv + b_qkv ; q,k,v = split(qkv)
  ag  = agent @ W_agent + b_agent ; q_agent,k_agent = split(ag)
  attn1 = softmax(q_agent @ k^T * s)        # [K, N]
  va    = (attn1 @ v) @ W_fc1 + b_fc1       # [K, d]
  attn2 = softmax(q @ k_agent^T * s)        # [N, K]
  out   = (attn2 @ va) @ W_fc2 + b_fc2 + x  # [N, d]

Host-side algebraic folds (everything not involving x is an input):
  q_agent/k_agent computed on host; q,k,v never materialized on device.
  S1^T = x @ (W_k@q_agent^T): b_k drops out of the softmax (shift invariance).
  va-chain: attn1 rows sum to 1, so all later biases fold into a single
       constant row bbig = (b_v@W_fc1+b_fc1)@W_fc2 + b_fc2 ADDED ON HOST.
  Wbig = W_v@W_fc1@W_fc2. Device:
    expS1[t,k] = exp(s*S1 - ln16)  (token-major, fp8)
    avxT[d,k]  = sum_t x[t,d]*expS1[t,k]   computed DIRECTLY transposed via
                 lhsT=xe-tile, rhs=expS1-tile DR matmuls (no PE transposes),
                 plus s1[k] = sum_t expS1[t,k] via a ones rhs.
    vaF2      = replicated [128,D] (1/s1)*(avxT^T @ Wbig): the vf matmul uses
                 column-duplicated avxT so both partition halves get vaF; 1/s1
                 is replicated to 128 partitions by an eye-stack matmul.
    expS2 packed [128, W]: chunk 2h on partitions 0:64 (DR matmul), chunk 2h+1
                 on partitions 64:128 (two non-DR matmuls; DR can't write a
                 partition-64 offset) -> one 128-partition exp per 1024 tokens
                 (halves the Act cost of stage-2 softmax).
    y[t,:]     = sum_k expS2[k,t]*vaF[k,:]   (256-col tiles, 2 per PSUM bank)
    yden[t]    = sum_k expS2[k,t]            (separate [128,1] matmuls)
  Host epilogue: out = y/yden + bbig + x  (exact fp32).

Scheduling notes (cost model): DMA_ENGINES is one serialized 360GB/s queue
(~6.5us in, ~2.9us out) and every engine SEQ is in-order, so (a) all
xT-dependent PE work (S1 + S2 matmuls) is emitted BEFORE the xe-dependent
avxT waves — otherwise the xe stream stalls every exp behind it; (b) xe is
streamed LAST so the final input byte feeds only the short avxT tail;
(c) wkq8 rides in the first xTw columns and fc2/feye share one param to
minimize DMA issue slots (HWDGE 625ns, SWDGE 1038ns on Pool, serialized);
(d) PSUM->SBUF casts only run on Act/DVE (cols*0.83+186 / cols*1.04+125 ns),
so y-phase casts are batched 4 tiles per instruction, alternating engines.
"""

import numpy as np
import ml_dtypes

B, N, D, K = 8, 4096, 256, 64
P = 128
NT = N // P        # 32 token tiles
DS = D // P        # 2 contraction subtiles
W = 512            # free-dim chunk for S2
NC2 = N // W       # 8 chunks
K2 = 2 * K         # wkq8 column block in xTw

XT_CHUNKS = [4, 12, 16]        # first chunk also carries wkq8
XE_HW = 8                      # leading xe tiles: HWDGE, issued after weights
XE_CHUNKS = [16, 8]            # rest: SWDGE, marker-gated past the xT tail
SLABS = [4, 6, 6, 8, 8]        # <=8 tiles (1 PSUM bank per slab)
YGROUPS = [4, 4, 4, 4, 4, 4, 4, 4]
YDMA = [8, 8, 8, 8]

_BF16 = ml_dtypes.bfloat16
_FP8 = ml_dtypes.float8_e4m3

_CACHE = {}


def _build_nc():
    import concourse.bass as bass
    import concourse.tile as tile
    from concourse import bacc, mybir

    f32 = mybir.dt.float32
    bf16 = mybir.dt.bfloat16
    fp8 = mybir.dt.float8e4
    Exp = mybir.ActivationFunctionType.Exp
    DR = mybir.MatmulPerfMode.DoubleRow
    Copy = mybir.ActivationFunctionType.Copy
    ts = bass.ts

    nc = bacc.Bacc("TRN2", target_bir_lowering=False, debug=False)

    # xTw: [wkq8 | x^T] so the first chunk DMA also delivers the projections
    xTw_d = nc.declare_dram_parameter("xTw", [P, DS, K2 + N], fp8, isOutput=False)
    xe_d = nc.declare_dram_parameter("xe", [P, NT, D], fp8, isOutput=False)
    wcombo_d = nc.declare_dram_parameter("wcombo", [P, DS, D], bf16, isOutput=False)
    # ff: exp-S2 bias (c2 - ln16, duplicated partition halves)
    ff_d = nc.declare_dram_parameter("ff", [P, 1], f32, isOutput=False)
    ye_d = nc.declare_dram_parameter("ye", [P, NT, D], fp8, isOutput=True)
    yden_d = nc.declare_dram_parameter("yden", [P, NT], bf16, isOutput=True)

    with tile.TileContext(nc) as tc:
        with (
            tc.tile_pool(name="sb", bufs=1) as sb,
            tc.tile_pool(name="yout", bufs=6) as yout,
        ):
            # ---------------- input DMAs ----------------
            xTw = sb.tile([P, DS, K2 + N], fp8)
            xe = sb.tile([P, NT, D], fp8)
            ff = sb.tile([P, 1], f32)
            wcombo = sb.tile([P, DS, D], bf16)
            wk = xTw[:, :, 0:K]
            wq = xTw[:, :, K:K2]

            def xt_cols(t0, t1):  # token-tile range -> xTw col range
                return xTw[:, :, K2 + P * t0 : K2 + P * t1]

            t0 = 0
            for ci, ct in enumerate(XT_CHUNKS):
                if ci == 0:
                    nc.sync.dma_start(
                        out=xTw[:, :, 0 : K2 + P * ct],
                        in_=xTw_d[:, :, 0 : K2 + P * ct],
                    )
                else:
                    nc.sync.dma_start(
                        out=xt_cols(t0, t0 + ct),
                        in_=xTw_d[:, :, K2 + P * t0 : K2 + P * (t0 + ct)],
                    )
                t0 += ct
            nc.sync.dma_start(out=ff, in_=ff_d[:, :])
            nc.sync.dma_start(out=wcombo, in_=wcombo_d[:, :, :])
            nc.sync.dma_start(out=xe[:, 0:XE_HW, :], in_=xe_d[:, 0:XE_HW, :])
            bias2 = ff[:, 0:1]

            expS1 = sb.tile([P, NT, K], fp8)    # token-major exp(S1), /16-shifted
            sh1 = sb.tile([P, 1], f32)
            nc.vector.memset(sh1, -2.772588722239781)   # -ln(16): keeps exp < 240 (fp8 max)
            ones8 = sb.tile([P, 2, 1], fp8)
            nc.vector.memset(ones8, 1.0)
            onesb = sb.tile([P, 1], bf16)
            nc.vector.memset(onesb, 1.0)
            # dummy exp: pulls the 1.3us LoadActFuncSet into the DMA head
            warm = sb.tile([P, 1], f32)
            nc.scalar.activation(warm, sh1, Exp)
            # packed exp(S2): chunk 2h on partitions 0:64, chunk 2h+1 on 64:128
            expS2 = sb.tile([P, NC2 // 2, W], bf16)

            vaF2 = sb.tile([P, D], bf16)        # replicated on both halves
            avxT2 = sb.tile([P, DS, K], bf16)
            rec2 = sb.tile([P, 1], f32)

            # ---- phase 1 ----
            with (
                tc.tile_pool(name="s1p", bufs=3, space="PSUM") as s1p,  # 3 banks
                tc.tile_pool(name="s2p", bufs=2, space="PSUM") as s2p,  # 2 banks
                tc.tile_pool(name="pX", bufs=1, space="PSUM") as pX,    # 1 bank
            ):
                # avxT accumulator [P, DS, K] plus s1 sums in one bank.
                # den is accumulated on BOTH partition halves ([0:64] via a
                # DoubleRow ones-matmul, [64:128] via non-DR single-tile
                # ones-matmuls, since DR can't write a partition-64 offset) so
                # 1/s1 comes out replicated with a single reciprocal.
                px = pX.tile([P, DS, K + 2], f32, tag="pX")
                den_a = px[0:K, 0, K : K + 1]           # [64, 1] partitions 0:64
                den_b = px[K:P, 0, K : K + 1]           # [64, 1] partitions 64:128
                den2 = px[:, 0, K : K + 1]              # [128, 1]
                nslab = len(SLABS)
                sstart = [sum(SLABS[:i]) for i in range(nslab)]

                def s1_slab(b):
                    t0, sl = sstart[b], SLABS[b]
                    ps = s1p.tile([P, sl, K], f32, tag="s1p")
                    for j in range(sl):
                        t = t0 + j
                        # DoubleRow: 2 fp8 weights/cell -> 256-contraction mm
                        nc.tensor.matmul(
                            ps[:, j, :], xt_cols(t, t + 1), wk,
                            start=True, stop=True, perf_mode=DR,
                        )
                    nc.scalar.activation(
                        expS1[:, t0 : t0 + sl, :], ps, Exp,
                        scale=float(D ** -0.5), bias=sh1,
                    )
                    return ps

                def xe_stream(marker_ps):
                    # trailing xe on SWDGE (parallel issue pipe), but each
                    # chunk's DMA overwrites a marker column copied from
                    # slab-0's PSUM: the WAW dep delays the Pool descriptor-
                    # gen until S1 has started, and the serial 1038ns gens
                    # then pace the xe transfers to land right AFTER the xT
                    # tail on the FIFO DMA_ENGINES queue, not before it.
                    t0 = XE_HW
                    for ct in XE_CHUNKS:
                        nc.vector.tensor_copy(
                            xe[:, t0 : t0 + 1, 0:1], marker_ps[:, 0:1, 0:1]
                        )
                        nc.gpsimd.dma_start(
                            out=xe[:, t0 : t0 + ct, :],
                            in_=xe_d[:, t0 : t0 + ct, :],
                        )
                        t0 += ct

                def s2_pair(h):
                    # chunks 2h (partitions 0:64, DR) + 2h+1 (64:128, non-DR
                    # since DoubleRow can't write a partition-offset dst) in
                    # ONE bank -> one 128-partition exp per 1024 tokens
                    p2 = s2p.tile([P, W], f32, tag="s2p")
                    nc.tensor.matmul(
                        p2[0:K, :], wq, xt_cols(8 * h, 8 * h + 4),
                        start=True, stop=True, perf_mode=DR,
                    )
                    for s in range(DS):
                        nc.tensor.matmul(
                            p2[K:2 * K, :],
                            xTw[:, s, K:K2],
                            xTw[:, s, K2 + W * (2 * h + 1) : K2 + W * (2 * h + 2)],
                            start=(s == 0), stop=(s == DS - 1),
                        )
                    nc.scalar.activation(
                        expS2[:, h, :], p2, Exp,
                        scale=float(D ** -0.5), bias=bias2,
                    )

                # xT-dependent work, ordered by data arrival: slabs 0-2 + S2
                # pairs 0-1 need only xT chunks 0-1; slabs 3-4 + pairs 2-3
                # need chunk 2. Emission order = engine order (in-order SEQs).
                ps0 = s1_slab(0)
                xe_stream(ps0)
                s1_slab(1)
                s1_slab(2)
                s2_pair(0)
                s2_pair(1)
                s1_slab(3)
                s1_slab(4)
                s2_pair(2)
                s2_pair(3)

                # --- xe-dependent avxT accumulation waves (PE after S1/S2) ---
                for u in range(NT // 2):
                    st, sp = (u == 0), (u == NT // 2 - 1)
                    e1u = expS1[:, 2 * u : 2 * u + 2, :]
                    for s in range(DS):
                        nc.tensor.matmul(
                            px[:, s, 0:K],
                            xe[:, 2 * u : 2 * u + 2, ts(s, P)],
                            e1u, start=st, stop=sp, perf_mode=DR,
                        )
                    nc.tensor.matmul(
                        den_a, e1u, ones8, start=st, stop=sp, perf_mode=DR,
                    )
                    for j in range(2):
                        t = 2 * u + j
                        nc.tensor.matmul(
                            den_b, expS1[:, t, :], ones8[:, 0, :],
                            start=(t == 0), stop=(t == NT - 1),
                        )

                # ---- vaF2 = rec2 * (avxT^T @ Wbig), replicated halves ----
                # vf is written twice (partitions 0:64 and 64:128, the second
                # via non-DR partition-offset dsts) from the SAME avxT lhsT,
                # so no column duplication or rec replication is needed. The
                # low half completes first and its scale runs on DVE while the
                # high half's scale runs on Act, so even-chunk y matmuls
                # (which read vaF2[0:64]) start as early as possible.
                nc.vector.reciprocal(rec2, den2)
                nc.vector.tensor_copy(avxT2[:, 0, :], px[:, 0, 0:K])
                nc.vector.tensor_copy(avxT2[:, 1, :], px[:, 1, 0:K])
                vf_ps = s1p.tile([P, D], f32, tag="s1p")
                for co in (0, K):
                    for s in range(DS):
                        nc.tensor.matmul(
                            vf_ps[co : co + K, :], avxT2[:, s, :],
                            wcombo[:, s, :],
                            start=(s == 0), stop=(s == DS - 1),
                        )
                nc.vector.tensor_scalar_mul(
                    vaF2[0:K, :], vf_ps[0:K, :], rec2[0:K, :]
                )
                nc.scalar.activation(
                    vaF2[K:P, :], vf_ps[K:P, :], Copy, scale=rec2[K:P, :]
                )

            # ---- y[t,:] = sum_k expS2[k,t]*vaF[k,:]; yden[t] = sum_k expS2 ----
            with (
                tc.tile_pool(name="ypool", bufs=3, space="PSUM") as ypool,
                tc.tile_pool(name="ydp", bufs=1, space="PSUM") as ydp,
            ):
                yden_ps = ydp.tile([P, NT], f32, tag="ydp")
                g0 = 0
                y_sb = None
                dma_i = 0
                dma_fill = 0
                for gi, gsz in enumerate(YGROUPS):
                    yp = ypool.tile([P, 2, W], f32, tag="ypool")
                    if dma_fill == 0:
                        y_sb = yout.tile([P, YDMA[dma_i], D], fp8, tag="ysb")
                        ysb0 = g0
                    for j in range(gsz):
                        t = g0 + j
                        c = t // 4
                        co = (c % 2) * K
                        e2t = expS2[co : co + K, c // 2, ts(t % 4, P)]
                        nc.tensor.matmul(
                            yp[:, j // 2, (j % 2) * D : (j % 2) * D + D],
                            e2t, vaF2[co : co + K, :], start=True, stop=True,
                        )
                        nc.tensor.matmul(
                            yden_ps[:, t : t + 1], e2t, onesb[co : co + K, :],
                            start=True, stop=True,
                        )
                    dst = y_sb[:, g0 - ysb0 : g0 - ysb0 + gsz, :]
                    src = yp[:, 0 : (gsz + 1) // 2, 0 : min(gsz, 2) * D]
                    if gi % 2 == 0:
                        nc.scalar.activation(dst, src, Copy)
                    else:
                        nc.vector.tensor_copy(dst, src)
                    dma_fill += gsz
                    if dma_fill == YDMA[dma_i]:
                        # early output DMAs go via SWDGE (Pool is idle in the
                        # y phase) so the final low-latency HWDGE issues never
                        # queue behind earlier output DMAs
                        eng = nc.gpsimd if dma_i < len(YDMA) - 2 else nc.sync
                        eng.dma_start(
                            out=ye_d[:, ysb0 : ysb0 + dma_fill, :],
                            in_=y_sb[:, 0:dma_fill, :],
                        )
                        dma_i += 1
                        dma_fill = 0
                    g0 += gsz
                # yden on Act (keeps DVE free for the vaF chain / y casts)
                yden_sb = sb.tile([P, NT], bf16)
                nc.scalar.activation(yden_sb, yden_ps, Copy)
                nc.gpsimd.dma_start(out=yden_d[:, :], in_=yden_sb)

    nc.compile()
    return nc


def _get_nc():
    if "nc" not in _CACHE:
        _CACHE["nc"] = _build_nc()
    return _CACHE["nc"]


def _prepare_in_maps(agent, x, W_qkv, b_qkv, W_agent, b_agent, W_fc1, b_fc1, W_fc2, b_fc2):
    # ---- host folds (float64 for stability, cast down at the end) ----
    agent64 = np.asarray(agent, np.float64)
    Wqkv64 = np.asarray(W_qkv, np.float64)
    bqkv64 = np.asarray(b_qkv, np.float64)
    Wag64 = np.asarray(W_agent, np.float64)
    bag64 = np.asarray(b_agent, np.float64)
    Wf1 = np.asarray(W_fc1, np.float64)
    bf1 = np.asarray(b_fc1, np.float64)
    Wf2 = np.asarray(W_fc2, np.float64)
    bf2 = np.asarray(b_fc2, np.float64)

    ag = agent64 @ Wag64 + bag64
    q_agent, k_agent = ag[:, :D], ag[:, D:]
    W_q, W_k, W_v = Wqkv64[:, :D], Wqkv64[:, D : 2 * D], Wqkv64[:, 2 * D :]
    b_q, b_v = bqkv64[:D], bqkv64[2 * D :]

    wk_f = W_k @ q_agent.T                      # [D, K]
    wq_f = W_q @ k_agent.T                      # [D, K]
    c2_f = (D ** -0.5) * (k_agent @ b_q)        # [K]
    Wbig = W_v @ Wf1 @ Wf2                      # [D, D]
    bbig = (b_v @ Wf1 + bf1) @ Wf2 + bf2        # [D], added on host

    # [D, D] -> [P, DS, D] with d = s*128 + p
    wcombo_b = np.ascontiguousarray(
        Wbig.reshape(DS, P, D).transpose(1, 0, 2)
    ).astype(_BF16)
    wkq8 = np.concatenate([wk_f, wq_f], axis=1).reshape(DS, P, K2)
    wkq8 = np.ascontiguousarray(wkq8.transpose(1, 0, 2)).astype(_FP8)
    ff = np.ascontiguousarray(
        np.tile(c2_f - 2.772588722239781, 2)[:, None]
    ).astype(np.float32)

    x32 = np.asarray(x, np.float32)
    # xe pack: [B, N, D] -> [B, P, NT, D], token = t*128 + p
    xeb = np.ascontiguousarray(
        x32.astype(_FP8).reshape(B, NT, P, D).transpose(0, 2, 1, 3)
    )
    # xTw pack: [wkq8 | x^T] -> [B, P, DS, K2 + N], d = s*128 + p
    xTb = x32.transpose(0, 2, 1).reshape(B, DS, P, N).transpose(0, 2, 1, 3)
    xTwb = np.empty((B, P, DS, K2 + N), _FP8)
    xTwb[:, :, :, 0:K2] = wkq8[None]
    xTwb[:, :, :, K2:] = xTb.astype(_FP8)

    in_maps = [
        {
            "xTw": xTwb[i],
            "xe": xeb[i],
            "wcombo": wcombo_b,
            "ff": ff,
        }
        for i in range(B)
    ]

    return in_maps, x32, bbig.astype(np.float32)


def kernel(**inputs):
    from concourse.bass_utils import run_bass_kernel_spmd

    in_maps, x32, bbig = _prepare_in_maps(**inputs)
    nc = _get_nc()
    res_obj = run_bass_kernel_spmd(nc, in_maps, core_ids=list(range(B)))
    _CACHE["last_results"] = res_obj
    res = res_obj.results

    # ye [P, NT, D] -> [N, D] with token = t*128 + p
    ye = np.stack(
        [np.asarray(res[i]["ye"]).transpose(1, 0, 2).reshape(N, D) for i in range(B)]
    ).astype(np.float32)
    yden = np.stack(
        [np.asarray(res[i]["yden"]).transpose(1, 0).reshape(N) for i in range(B)]
    ).astype(np.float32)
    out = ye / yden[:, :, None] + bbig[None, None, :] + x32
    return out.astype(np.float32)
